# revision 1
# baseline (speedup 1.0000x reference)
"""Bahdanau-attention scoring kernel for Trainium2 (8 NeuronCores).

reference computation:
  enc = transpose(encoderOutputs, (1,0,2))            # [B,S,H]
  energy = tanh(concat([hidden bcast, enc]) @ W^T(2H contraction) + b)
  scores = energy . v ; softmax over S -> [B,1,S]

decomposition used here:
  energy[b,s,h] = tanh( enc[b,s,:] @ W2[h,:] + (hidden[b,:] @ W1[h,:] + b[h]) )
  with W1 = W[:, :H], W2 = W[:, H:].
  The hidden term ("ubias") is per-(b,h), computed once on-device, and folded
  into the tanh as the ScalarE activation's per-partition bias.

sharding: data-parallel over batch B=32 -> 4 batches per core.
Per-core kernel layout:
  - energy tiles [h=128 part, rows=512 free] via fp32r matmuls
    (lhsT = W2T k-chunk x h-chunk, rhs = encT k-chunk x row-block)
  - tanh fused with per-partition ubias on ScalarE
  - v-dot: DVE accumulates acc += tanh_chunk * v_chunk per h-chunk, then one
    matmul per row-block with a one-hot ones column reduces partitions and
    lands batch bb's scores on psum partition 32*bb (engine partition bases
    must be 32-aligned)
  - softmax over S on a [128, 2048] sbuf tile (4 used partitions), out f32

toolchain notes (this container):
  - walrus here accepts only ONE sync wait per instruction; _split_multiwaits
    rewrites the BIR to single-wait NoOp chains (hooked via nc.to_json_bytes)
  - fp32r matmuls need fp32r-declared producers; inputs are pre-rounded on
    the host (RNE to the fp32r grid) and declared float32r in DRAM
"""

import json
import sys
import types

import numpy as np

H = 1024
S = 2048
B = 32
NCORES = 8
B_LOC = B // NCORES          # 4 batches per core
R = S * B_LOC                # 8192 rows per core (b-major: r = b*S + s)
NBLK = R // 512              # 16 row blocks of 512
KC = H // 128                # 8 contraction chunks
HC = H // 128                # 8 h chunks


def _install_ntff_hook():
    """Install antenv.axon_hooks shim so trace=True works under axon."""
    if "antenv.axon_hooks" in sys.modules:
        return
    try:
        from trn_agent_boot.trn_boot import _ntff_profile_via_ctypes

        hook = _ntff_profile_via_ctypes("/opt/axon/libaxon_pjrt.so")
    except Exception:
        hook = None
    mod = types.ModuleType("antenv.axon_hooks")
    mod._hook = hook
    mod.get_axon_ntff_profile_hook = lambda: mod._hook

    def _set(h):
        mod._hook = h

    mod.set_axon_ntff_profile_hook = _set
    sys.modules["antenv.axon_hooks"] = mod


def _split_multiwaits(bir):
    """This walrus build supports one sync wait per instruction: split
    longer on_wait lists into single-wait NoOps on the same engine."""
    for fn in bir["functions"]:
        for blk in fn["blocks"]:
            out = []
            for inst in blk["instructions"]:
                si = inst.get("sync_info")
                ow = (si or {}).get("on_wait") or []
                if len(ow) > 1:
                    for j, w in enumerate(ow[:-1]):
                        out.append(
                            {
                                "debug": inst.get("debug", 0),
                                "engine": inst["engine"],
                                "ins": [],
                                "name": f"{inst['name']}_sw{j}",
                                "opcode": "NoOp",
                                "outs": [],
                                "sync_info": {"on_wait": [w], "on_update": []},
                                "text_hint": "waitsplit",
                            }
                        )
                    si["on_wait"] = [ow[-1]]
                out.append(inst)
            blk["instructions"] = out
    return bir


def _patch_json(nc):
    orig = nc.to_json_bytes

    def patched():
        return json.dumps(_split_multiwaits(json.loads(orig()))).encode()

    nc.to_json_bytes = patched


def build_kernel():
    import concourse.bass as bass
    import concourse.tile as tile
    from concourse import mybir
    from concourse.masks import make_identity

    f32 = mybir.dt.float32
    f32r = mybir.dt.float32r
    AF = mybir.ActivationFunctionType

    nc = bass.Bass("TRN2", target_bir_lowering=False, debug=False, num_devices=1)

    # fp32r-consumed inputs are declared float32r; the host pre-rounds their
    # values (fp32r = fp32 with mantissa RNE-rounded, 12 low bits dropped).
    enc_t = nc.dram_tensor("enc_t", [H, R], f32r, kind="ExternalInput").ap()
    h_t = nc.dram_tensor("h_t", [128, KC * B_LOC], f32r, kind="ExternalInput").ap()
    w1t = nc.dram_tensor("w1t", [H, H], f32r, kind="ExternalInput").ap()
    w2t = nc.dram_tensor("w2t", [H, H], f32r, kind="ExternalInput").ap()
    bcol = nc.dram_tensor("bcol", [128, HC], f32, kind="ExternalInput").ap()
    vcol = nc.dram_tensor("vcol", [128, HC], f32, kind="ExternalInput").ap()
    onesoh = nc.dram_tensor("onesoh", [128, B_LOC * 128], f32r, kind="ExternalInput").ap()
    out = nc.dram_tensor("out", [B_LOC, S], f32, kind="ExternalOutput").ap()

    with tile.TileContext(nc) as tc:
        with (
            tc.tile_pool(name="consts", bufs=1) as consts,
            tc.tile_pool(name="w1p", bufs=1) as w1p,
            tc.tile_pool(name="w2p", bufs=1) as w2p,
            tc.tile_pool(name="encp", bufs=2) as encp,
            tc.tile_pool(name="tanp", bufs=3) as tanp,
            tc.tile_pool(name="tmpp", bufs=2) as tmpp,
            tc.tile_pool(name="accp", bufs=2) as accp,
            tc.tile_pool(name="scorep", bufs=1) as scorep,
            tc.tile_pool(name="softp", bufs=1) as softp,
            tc.tile_pool(name="ep0", bufs=4, space="PSUM") as ep0,      # blk0 kc-outer
            tc.tile_pool(name="epsum", bufs=2, space="PSUM") as epsum,  # blks >= 1
            tc.tile_pool(name="spsum", bufs=2, space="PSUM") as spsum,  # scores + ubias
        ):
            # ---- W2T lower halves + enc block 0 first on the SP queue -----
            # (the first matmuls need w2sb[:, kc, 0:512] + et0[kc]; everything
            # small rides the gpsimd queue in parallel)
            w2sb = w2p.tile([128, KC, H], f32r, tag="w2sb")
            et0 = []
            for kc in range(KC):
                nc.sync.dma_start(
                    w2sb[:, kc, 0:512], w2t[kc * 128 : (kc + 1) * 128, 0:512]
                )
                et = encp.tile([128, 512], f32r, tag=f"enc{kc}")
                nc.sync.dma_start(et[:], enc_t[kc * 128 : (kc + 1) * 128, 0:512])
                et0.append(et)
            for kc in range(KC):
                nc.sync.dma_start(
                    w2sb[:, kc, 512:H], w2t[kc * 128 : (kc + 1) * 128, 512:H]
                )

            # ---- small constants on the gpsimd queue ----------------------
            h_sb = consts.tile([128, KC, B_LOC], f32r, tag="h_sb")
            nc.gpsimd.dma_start(h_sb[:], h_t.rearrange("p (c b) -> p c b", c=KC))
            bcol_sb = consts.tile([128, HC], f32, tag="bcol_sb")
            nc.gpsimd.dma_start(bcol_sb[:], bcol[:])
            vcol_sb = consts.tile([128, HC], f32, tag="vcol_sb")
            nc.gpsimd.dma_start(vcol_sb[:], vcol[:])

            # ones one-hot for the partition-sum matmul: column 32*bb is 1
            ones_oh = consts.tile([128, B_LOC, 128], f32r, tag="ones_oh")
            nc.gpsimd.dma_start(
                ones_oh[:], onesoh.rearrange("p (b m) -> p b m", b=B_LOC)
            )

            # ---- W1T resident like W2T, on the ACT HWDGE queue ------------
            w1sb = w1p.tile([128, KC, H], f32r, tag="w1sb")
            for kc in range(KC):
                nc.scalar.dma_start(w1sb[:, kc, :], w1t[kc * 128 : (kc + 1) * 128, :])

            # 4x4 identity for the tiny PE transposes of uT
            idt = consts.tile([B_LOC, B_LOC], f32, tag="idt")
            make_identity(nc, idt[:])

            # uT[b, h] = (hidden @ W1^T)[b, h] via wide-N matmuls with the
            # 4-column hidden as the stationary operand (cheap weight loads),
            # then 8 tiny PE transposes to get ubias in [h-part, b] layout
            uts = consts.tile([B_LOC, H], f32, tag="uts")
            ubias = consts.tile([128, HC, B_LOC], f32, tag="ubias")

            def emit_u_half(nh):
                upt = spsum.tile([128, 512], f32, tag="sp")
                for kc in range(KC):
                    nc.tensor.matmul(
                        upt[0:B_LOC, :],
                        h_sb[:, kc, :],
                        w1sb[:, kc, nh * 512 : (nh + 1) * 512],
                        start=(kc == 0),
                        stop=(kc == KC - 1),
                        skip_group_check=True,
                    )
                nc.vector.tensor_copy(
                    uts[0:B_LOC, nh * 512 : (nh + 1) * 512], upt[0:B_LOC, :]
                )
                for hc in range(nh * 4, nh * 4 + 4):
                    trp = spsum.tile([128, 512], f32, tag="sp")
                    nc.tensor.transpose(
                        trp[:, 0:B_LOC],
                        uts[0:B_LOC, hc * 128 : (hc + 1) * 128],
                        idt[:],
                    )
                    nc.vector.tensor_scalar_add(
                        ubias[:, hc, :], trp[:, 0:B_LOC], bcol_sb[:, hc : hc + 1]
                    )

            # ---- main loop over 16 row blocks -----------------------------
            # batch bb's scores live on partition 32*bb
            scores = scorep.tile([128, S], f32, tag="scores")
            nc.vector.memset(scores[:], 0.0)

            pending_sum = None  # (acc tile, bb, sb) awaiting partition-sum MM

            def emit_sum(pending):
                acc, bb, sb = pending
                mw = 32 * bb + 1
                sp = spsum.tile([128, 512], f32, tag="sp")
                nc.tensor.matmul(
                    sp[0:mw, :],
                    ones_oh[:, bb, 0:mw],
                    acc[:],
                    start=True,
                    stop=True,
                    skip_group_check=True,
                )
                nc.vector.tensor_copy(
                    scores[32 * bb : 32 * bb + 1, sb * 512 : (sb + 1) * 512],
                    sp[32 * bb : 32 * bb + 1, :],
                )

            for blk in range(NBLK):
                bb = blk // (S // 512)       # batch of this block
                sb = blk % (S // 512)        # block index within the batch
                if blk == 0:
                    etiles = et0
                else:
                    etiles = []
                    for kc in range(KC):
                        et = encp.tile([128, 512], f32r, tag=f"enc{kc}")
                        eng = nc.sync
                        eng.dma_start(
                            et[:],
                            enc_t[
                                kc * 128 : (kc + 1) * 128, blk * 512 : (blk + 1) * 512
                            ],
                        )
                        etiles.append(et)

                acc = accp.tile([128, 512], f32r, tag="acc")

                def postproc(ep, hc):
                    # tanh with fused ubias, then DVE v-scale + accumulate
                    tt = tanp.tile([128, 512], f32, tag="tt")
                    nc.scalar.activation(
                        tt[:], ep[:], AF.Tanh,
                        bias=ubias[:, hc, bb : bb + 1], scale=1.0,
                    )
                    if hc == 0:
                        nc.vector.tensor_scalar_mul(
                            acc[:], tt[:], vcol_sb[:, hc : hc + 1]
                        )
                    else:
                        tmp = tmpp.tile([128, 512], f32, tag="tmp")
                        nc.vector.tensor_scalar_mul(
                            tmp[:], tt[:], vcol_sb[:, hc : hc + 1]
                        )
                        nc.vector.tensor_tensor(
                            acc[:], tmp[:], acc[:], op=mybir.AluOpType.add
                        )

                if blk == 0:
                    # kc-outer halves: PE can start as soon as the first
                    # (w2, enc) pair lands instead of waiting for all 8
                    for half in range(2):
                        hcs = range(half * 4, half * 4 + 4)
                        eps = {}
                        for hc in hcs:
                            e0t = ep0.tile([128, 512], f32, tag="e0")
                            eps[hc] = e0t
                        for kc in range(KC):
                            for hc in hcs:
                                nc.tensor.matmul(
                                    eps[hc][:],
                                    w2sb[:, kc, hc * 128 : (hc + 1) * 128],
                                    etiles[kc][:],
                                    start=(kc == 0),
                                    stop=(kc == KC - 1),
                                    skip_group_check=True,
                                )
                        emit_u_half(half)
                        for hc in hcs:
                            postproc(eps[hc], hc)
                else:
                    for hc in range(HC):
                        ep = epsum.tile([128, 512], f32, tag="ep")
                        last_chunk = blk == NBLK - 1 and hc == HC - 1
                        if last_chunk:
                            # split the very last energy group into two
                            # 256-column halves so the tanh/v-dot chain (and
                            # with it the final partition-sum) starts half a
                            # group earlier - trims the end-of-stream stall
                            tt_l = tanp.tile([128, 512], f32, tag="tt")
                            tmp_l = tmpp.tile([128, 512], f32, tag="tmp")
                            for half in range(2):
                                sl = slice(half * 256, half * 256 + 256)
                                for kc in range(KC):
                                    nc.tensor.matmul(
                                        ep[:, sl],
                                        w2sb[:, kc, hc * 128 : (hc + 1) * 128],
                                        etiles[kc][:, sl],
                                        start=(kc == 0),
                                        stop=(kc == KC - 1),
                                        skip_group_check=True,
                                    )
                                nc.scalar.activation(
                                    tt_l[:, sl], ep[:, sl], AF.Tanh,
                                    bias=ubias[:, hc, bb : bb + 1], scale=1.0,
                                )
                                nc.vector.tensor_scalar_mul(
                                    tmp_l[:, sl], tt_l[:, sl],
                                    vcol_sb[:, hc : hc + 1],
                                )
                                nc.vector.tensor_tensor(
                                    acc[:, sl], tmp_l[:, sl], acc[:, sl],
                                    op=mybir.AluOpType.add,
                                )
                            continue
                        for kc in range(KC):
                            nc.tensor.matmul(
                                ep[:],
                                w2sb[:, kc, hc * 128 : (hc + 1) * 128],
                                etiles[kc][:],
                                start=(kc == 0),
                                stop=(kc == KC - 1),
                            )
                        if hc == 0 and pending_sum is not None:
                            emit_sum(pending_sum)
                            pending_sum = None
                        postproc(ep, hc)

                pending_sum = (acc, bb, sb)

            emit_sum(pending_sum)

            # ---- softmax over S per batch ---------------------------------
            # no max-subtraction: |scores| <= ||v||_1 (~25), exp() is safely
            # inside fp32 range, and softmax is shift-invariant
            esc = softp.tile([128, S], f32, tag="esc")
            ssum = softp.tile([128, 1], f32, tag="ssum")
            nc.scalar.activation(
                esc[:], scores[:], AF.Exp, bias=0.0, scale=1.0,
                accum_out=ssum[:],
            )
            rsum = softp.tile([128, 1], f32, tag="rsum")
            nc.vector.reciprocal(rsum[:], ssum[:])
            prob = softp.tile([128, S], f32, tag="prob")
            nc.vector.tensor_scalar_mul(prob[:], esc[:], rsum[:])
            for bb in range(B_LOC):
                nc.sync.dma_start(
                    out[bb : bb + 1, :], prob[32 * bb : 32 * bb + 1, :]
                )

    _patch_json(nc)
    return nc


_NC_CACHE = None


def _get_nc():
    global _NC_CACHE
    if _NC_CACHE is None:
        _NC_CACHE = build_kernel()
    return _NC_CACHE


def round_fp32r(x):
    """RNE-round fp32 values to the fp32r grid (drop low 12 mantissa bits)."""
    u = np.ascontiguousarray(x, dtype=np.float32).view(np.uint32)
    r = (u + 0x7FF + ((u >> 12) & 1)) & 0xFFFFF000
    return r.astype(np.uint32).view(np.float32)


def shard_inputs(hidden, encoderOutputs, W, b, v):
    """Host-side prep: per-core input dict list."""
    hidden = np.ascontiguousarray(hidden, dtype=np.float32)
    W = np.ascontiguousarray(W, dtype=np.float32)
    b = np.ascontiguousarray(b, dtype=np.float32)
    v = np.ascontiguousarray(v, dtype=np.float32)

    w1t = round_fp32r(np.ascontiguousarray(W[:, :H].T))  # [k, h]
    w2t = round_fp32r(np.ascontiguousarray(W[:, H:].T))  # [k, h]
    bcol = np.ascontiguousarray(b.reshape(HC, 128).T)    # [128, hc]
    vcol = np.ascontiguousarray(v.reshape(HC, 128).T)   # [128, hc]
    onesoh = np.zeros((128, B_LOC, 128), np.float32)
    for bb in range(B_LOC):
        onesoh[:, bb, 32 * bb] = 1.0
    onesoh = np.ascontiguousarray(onesoh.reshape(128, B_LOC * 128))

    # [H, B, S] single big transpose, then per-core contiguous slices
    encT = round_fp32r(
        np.transpose(np.asarray(encoderOutputs, dtype=np.float32), (2, 1, 0))
    )

    in_maps = []
    for i in range(NCORES):
        b0 = i * B_LOC
        enc_c = np.ascontiguousarray(encT[:, b0 : b0 + B_LOC, :]).reshape(H, R)
        hid_c = hidden[b0 : b0 + B_LOC]                  # [4, H]
        h_t = round_fp32r(
            hid_c.T.reshape(KC, 128, B_LOC).transpose(1, 0, 2).reshape(128, KC * B_LOC)
        )
        in_maps.append(
            {
                "enc_t": enc_c,
                "h_t": h_t,
                "w1t": w1t,
                "w2t": w2t,
                "bcol": bcol,
                "vcol": vcol,
                "onesoh": onesoh,
            }
        )
    return in_maps


def run(in_maps, trace=False):
    if trace:
        _install_ntff_hook()
    from concourse import bass_utils

    nc = _get_nc()
    res = bass_utils.run_bass_kernel_spmd(
        nc, in_maps, core_ids=list(range(NCORES)), trace=trace
    )
    return res


def kernel(hidden, encoderOutputs, W, b, v):
    in_maps = shard_inputs(hidden, encoderOutputs, W, b, v)
    res = run(in_maps, trace=False)
    outs = [res.results[i]["out"] for i in range(NCORES)]   # each [4, S]
    full = np.concatenate(outs, axis=0)                     # [32, S]
    return full[:, None, :].astype(np.float32)              # [32, 1, S]



# revision 2
# speedup vs baseline: 1.0244x; 1.0244x over previous
"""Bahdanau-attention scoring kernel for Trainium2 (8 NeuronCores).

reference computation:
  enc = transpose(encoderOutputs, (1,0,2))            # [B,S,H]
  energy = tanh(concat([hidden bcast, enc]) @ W^T(2H contraction) + b)
  scores = energy . v ; softmax over S -> [B,1,S]

decomposition used here:
  energy[b,s,h] = tanh( enc[b,s,:] @ W2[h,:] + (hidden[b,:] @ W1[h,:] + b[h]) )
  with W1 = W[:, :H], W2 = W[:, H:].
  The hidden term ("ubias") is per-(b,h), computed once on-device, and folded
  into the tanh as the ScalarE activation's per-partition bias.

fp8 main GEMM:
  enc is quantized to fp8 e4m3 (x16 scale); W2 is split into TWO e4m3 terms
  at a common x32 scale (w2hi = q(32*W2), w2lo = q(32*W2 - w2hi)) so W2's
  quantization noise cancels; only enc's ~1.8% element noise remains
  (measured end-to-end rel_fro ~1.5e-2 vs the 2e-2 gate).
  Matmuls run in MatmulPerfMode.DoubleRow: operands are [128, 2, N] APs
  (two k-chunks per instruction) at 0.5 cycles/row. The x512 scale is
  undone by the tanh activation's scale (tanh(psum/512 + ubias)).

sharding: data-parallel over batch B=32 -> 4 batches per core.
Per-core kernel layout:
  - energy tiles [h=128 part, rows=512 free] via fp8 DoubleRow matmuls
    (8 per tile: 4 kc-pairs x {hi, lo})
  - tanh fused with per-partition ubias on ScalarE, output bf16
  - v-dot: one fused DVE scalar_tensor_tensor per h-chunk
    (acc = tanh*v_chunk + acc), then one fp32r matmul per row-block with a
    one-hot ones column reduces partitions and lands batch bb's scores on
    psum partition 32*bb
  - softmax over S on a [128, 2048] sbuf tile (4 used partitions), out f32

toolchain notes (this container):
  - walrus here accepts only ONE sync wait per instruction; _split_multiwaits
    rewrites the BIR to single-wait NoOp chains (hooked via nc.to_json_bytes)
  - fp32r matmuls need fp32r-declared producers; inputs are pre-rounded on
    the host (RNE to the fp32r grid) and declared float32r in DRAM
"""

import json
import sys
import types

import ml_dtypes
import numpy as np

H = 1024
S = 2048
B = 32
NCORES = 8
B_LOC = B // NCORES          # 4 batches per core
R = S * B_LOC                # 8192 rows per core (b-major: r = b*S + s)
NBLK = R // 512              # 16 row blocks of 512
KC = H // 128                # 8 contraction chunks
HC = H // 128                # 8 h chunks

S_E = 16.0                   # enc fp8 scale
S_W = 32.0                   # W2 fp8 scale
ACT_SCALE = 1.0 / (S_E * S_W)

F8 = ml_dtypes.float8_e4m3


def _install_ntff_hook():
    """Install antenv.axon_hooks shim so trace=True works under axon."""
    if "antenv.axon_hooks" in sys.modules:
        return
    try:
        from trn_agent_boot.trn_boot import _ntff_profile_via_ctypes

        hook = _ntff_profile_via_ctypes("/opt/axon/libaxon_pjrt.so")
    except Exception:
        hook = None
    mod = types.ModuleType("antenv.axon_hooks")
    mod._hook = hook
    mod.get_axon_ntff_profile_hook = lambda: mod._hook

    def _set(h):
        mod._hook = h

    mod.set_axon_ntff_profile_hook = _set
    sys.modules["antenv.axon_hooks"] = mod


def _split_multiwaits(bir):
    """This walrus build supports one sync wait per instruction: split
    longer on_wait lists into single-wait NoOps on the same engine."""
    for fn in bir["functions"]:
        for blk in fn["blocks"]:
            out = []
            for inst in blk["instructions"]:
                si = inst.get("sync_info")
                ow = (si or {}).get("on_wait") or []
                if len(ow) > 1:
                    for j, w in enumerate(ow[:-1]):
                        out.append(
                            {
                                "debug": inst.get("debug", 0),
                                "engine": inst["engine"],
                                "ins": [],
                                "name": f"{inst['name']}_sw{j}",
                                "opcode": "NoOp",
                                "outs": [],
                                "sync_info": {"on_wait": [w], "on_update": []},
                                "text_hint": "waitsplit",
                            }
                        )
                    si["on_wait"] = [ow[-1]]
                out.append(inst)
            blk["instructions"] = out
    return bir


def _patch_json(nc):
    orig = nc.to_json_bytes

    def patched():
        return json.dumps(_split_multiwaits(json.loads(orig()))).encode()

    nc.to_json_bytes = patched


def build_kernel():
    import concourse.bass as bass
    import concourse.tile as tile
    from concourse import mybir
    from concourse.masks import make_identity

    f32 = mybir.dt.float32
    f32r = mybir.dt.float32r
    bf16 = mybir.dt.bfloat16
    f8 = mybir.dt.float8e4
    AF = mybir.ActivationFunctionType
    DR = mybir.MatmulPerfMode.DoubleRow
    MUL = mybir.AluOpType.mult
    ADD = mybir.AluOpType.add

    nc = bass.Bass("TRN2", target_bir_lowering=False, debug=False, num_devices=1)

    # All big operands are stored partition-major in DRAM ([128, chunks*cols])
    # so each SBUF tile fills with a single (or few) 3D DMA.
    enc_t = nc.dram_tensor("enc_t", [128, KC * R], f8, kind="ExternalInput").ap()
    h_t = nc.dram_tensor("h_t", [128, KC * B_LOC], f32r, kind="ExternalInput").ap()
    w1t = nc.dram_tensor("w1t", [128, KC * H], f32r, kind="ExternalInput").ap()
    w2hi = nc.dram_tensor("w2hi", [128, KC * H], f8, kind="ExternalInput").ap()
    w2lo = nc.dram_tensor("w2lo", [128, KC * H], f8, kind="ExternalInput").ap()
    bcol = nc.dram_tensor("bcol", [128, HC], f32, kind="ExternalInput").ap()
    vcol = nc.dram_tensor("vcol", [128, HC], f32, kind="ExternalInput").ap()
    onesoh = nc.dram_tensor("onesoh", [128, B_LOC * 128], f32r, kind="ExternalInput").ap()
    out = nc.dram_tensor("out", [B_LOC, S], f32, kind="ExternalOutput").ap()

    enc3 = enc_t.rearrange("p (c r) -> p c r", c=KC)
    w13 = w1t.rearrange("p (c h) -> p c h", c=KC)
    w2hi3 = w2hi.rearrange("p (c h) -> p c h", c=KC)
    w2lo3 = w2lo.rearrange("p (c h) -> p c h", c=KC)

    with tile.TileContext(nc) as tc:
        with (
            tc.tile_pool(name="consts", bufs=1) as consts,
            tc.tile_pool(name="w1p", bufs=1) as w1p,
            tc.tile_pool(name="w2p", bufs=1) as w2p,
            tc.tile_pool(name="encp", bufs=3) as encp,
            tc.tile_pool(name="tanp", bufs=3) as tanp,
            tc.tile_pool(name="accp", bufs=2) as accp,
            tc.tile_pool(name="scorep", bufs=1) as scorep,
            tc.tile_pool(name="softp", bufs=1) as softp,
            tc.tile_pool(name="ep0", bufs=4, space="PSUM") as ep0,      # blk0 pair-outer
            tc.tile_pool(name="epsum", bufs=2, space="PSUM") as epsum,  # blks >= 1
            tc.tile_pool(name="spsum", bufs=2, space="PSUM") as spsum,  # scores + ubias
        ):
            # ---- W2 hi/lo + enc block 0 on the SP queue ------------------
            # first kc-pair of everything first so the pair-outer matmuls of
            # block 0 can start while the rest streams in
            w2hi_sb = w2p.tile([128, KC, H], f8, tag="w2hi_sb")
            w2lo_sb = w2p.tile([128, KC, H], f8, tag="w2lo_sb")
            et0 = encp.tile([128, KC, 512], f8, tag="enc")
            nc.sync.dma_start(w2hi_sb[:, 0:2, :], w2hi3[:, 0:2, :])
            nc.sync.dma_start(w2lo_sb[:, 0:2, :], w2lo3[:, 0:2, :])
            nc.sync.dma_start(et0[:, 0:2, :], enc3[:, 0:2, 0:512])
            nc.sync.dma_start(w2hi_sb[:, 2:KC, :], w2hi3[:, 2:KC, :])
            nc.sync.dma_start(w2lo_sb[:, 2:KC, :], w2lo3[:, 2:KC, :])
            nc.sync.dma_start(et0[:, 2:KC, :], enc3[:, 2:KC, 0:512])

            # ---- small constants on the gpsimd queue ----------------------
            h_sb = consts.tile([128, KC, B_LOC], f32r, tag="h_sb")
            nc.gpsimd.dma_start(h_sb[:], h_t.rearrange("p (c b) -> p c b", c=KC))
            bcol_sb = consts.tile([128, HC], f32, tag="bcol_sb")
            nc.gpsimd.dma_start(bcol_sb[:], bcol[:])
            vcol_sb = consts.tile([128, HC], f32, tag="vcol_sb")
            nc.gpsimd.dma_start(vcol_sb[:], vcol[:])

            # ones one-hot for the partition-sum matmul: column 32*bb is 1
            ones_oh = consts.tile([128, B_LOC, 128], f32r, tag="ones_oh")
            nc.gpsimd.dma_start(
                ones_oh[:], onesoh.rearrange("p (b m) -> p b m", b=B_LOC)
            )

            # ---- W1T resident, one DMA on the ACT HWDGE queue -------------
            w1sb = w1p.tile([128, KC, H], f32r, tag="w1sb")
            nc.scalar.dma_start(w1sb[:], w13[:])

            # 4x4 identity for the tiny PE transposes of uT
            idt = consts.tile([B_LOC, B_LOC], f32, tag="idt")
            make_identity(nc, idt[:])

            # uT[b, h] = (hidden @ W1^T)[b, h] via wide-N matmuls with the
            # 4-column hidden as the stationary operand (cheap weight loads),
            # then 8 tiny PE transposes to get ubias in [h-part, b] layout
            uts = consts.tile([B_LOC, H], f32, tag="uts")
            ubias = consts.tile([128, HC, B_LOC], f32, tag="ubias")

            def emit_u_half(nh):
                upt = spsum.tile([128, 512], f32, tag="sp")
                for kc in range(KC):
                    nc.tensor.matmul(
                        upt[0:B_LOC, :],
                        h_sb[:, kc, :],
                        w1sb[:, kc, nh * 512 : (nh + 1) * 512],
                        start=(kc == 0),
                        stop=(kc == KC - 1),
                        skip_group_check=True,
                    )
                nc.vector.tensor_copy(
                    uts[0:B_LOC, nh * 512 : (nh + 1) * 512], upt[0:B_LOC, :]
                )
                for hc in range(nh * 4, nh * 4 + 4):
                    trp = spsum.tile([128, 512], f32, tag="sp")
                    nc.tensor.transpose(
                        trp[:, 0:B_LOC],
                        uts[0:B_LOC, hc * 128 : (hc + 1) * 128],
                        idt[:],
                    )
                    nc.vector.tensor_scalar_add(
                        ubias[:, hc, :], trp[:, 0:B_LOC], bcol_sb[:, hc : hc + 1]
                    )

            # ---- main loop over 16 row blocks -----------------------------
            # batch bb's scores live on partition 32*bb
            scores = scorep.tile([128, S], f32, tag="scores")
            nc.vector.memset(scores[:], 0.0)

            pending_sum = None  # (acc tile, bb, sb) awaiting partition-sum MM

            def emit_sum(pending):
                acc, bb, sb = pending
                mw = 32 * bb + 1
                sp = spsum.tile([128, 512], f32, tag="sp")
                nc.tensor.matmul(
                    sp[0:mw, :],
                    ones_oh[:, bb, 0:mw],
                    acc[:],
                    start=True,
                    stop=True,
                    skip_group_check=True,
                )
                nc.vector.tensor_copy(
                    scores[32 * bb : 32 * bb + 1, sb * 512 : (sb + 1) * 512],
                    sp[32 * bb : 32 * bb + 1, :],
                )

            ets = {0: et0}
            for blk in range(NBLK):
                bb = blk // (S // 512)       # batch of this block
                sb = blk % (S // 512)        # block index within the batch
                if blk + 1 < NBLK:
                    etn = encp.tile([128, KC, 512], f8, tag="enc")
                    nc.sync.dma_start(
                        etn[:], enc3[:, :, (blk + 1) * 512 : (blk + 2) * 512]
                    )
                    ets[blk + 1] = etn
                et = ets.pop(blk)

                acc = accp.tile([128, 512], f32r, tag="acc")

                def postproc(ep, hc):
                    # tanh with fused ubias (undoes the x512 fp8 scale), then
                    # one fused DVE op: acc = tanh*v_chunk (+ acc)
                    tt = tanp.tile([128, 512], bf16, tag="tt")
                    nc.scalar.activation(
                        tt[:], ep[:], AF.Tanh,
                        bias=ubias[:, hc, bb : bb + 1], scale=ACT_SCALE,
                    )
                    if hc == 0:
                        nc.vector.tensor_scalar_mul(
                            acc[:], tt[:], vcol_sb[:, hc : hc + 1]
                        )
                    else:
                        nc.vector.scalar_tensor_tensor(
                            acc[:], tt[:], vcol_sb[:, hc : hc + 1], acc[:],
                            op0=MUL, op1=ADD,
                        )

                if blk == 0:
                    # pair-outer halves: PE can start as soon as the first
                    # (w2, enc) kc-pair lands instead of waiting for all 8
                    for half in range(2):
                        hcs = range(half * 4, half * 4 + 4)
                        eps = {}
                        for hc in hcs:
                            e0t = ep0.tile([128, 512], f32, tag="e0")
                            eps[hc] = e0t
                        for p in range(4):
                            for hc in hcs:
                                hsl = slice(hc * 128, (hc + 1) * 128)
                                nc.tensor.matmul(
                                    eps[hc][:],
                                    w2hi_sb[:, 2 * p : 2 * p + 2, hsl],
                                    et[:, 2 * p : 2 * p + 2, :],
                                    start=(p == 0),
                                    stop=False,
                                    perf_mode=DR,
                                    skip_group_check=True,
                                )
                                nc.tensor.matmul(
                                    eps[hc][:],
                                    w2lo_sb[:, 2 * p : 2 * p + 2, hsl],
                                    et[:, 2 * p : 2 * p + 2, :],
                                    start=False,
                                    stop=(p == 3),
                                    perf_mode=DR,
                                    skip_group_check=True,
                                )
                        emit_u_half(half)
                        for hc in hcs:
                            postproc(eps[hc], hc)
                else:
                    for hc in range(HC):
                        hsl = slice(hc * 128, (hc + 1) * 128)
                        ep = epsum.tile([128, 512], f32, tag="ep")
                        last_chunk = blk == NBLK - 1 and hc == HC - 1
                        if last_chunk:
                            # split the very last energy group into two
                            # 256-column halves so the tanh/v-dot chain (and
                            # with it the final partition-sum) starts half a
                            # group earlier - trims the end-of-stream stall
                            tt_l = tanp.tile([128, 512], bf16, tag="tt")
                            for half in range(2):
                                sl = slice(half * 256, half * 256 + 256)
                                for p in range(4):
                                    nc.tensor.matmul(
                                        ep[:, sl],
                                        w2hi_sb[:, 2 * p : 2 * p + 2, hsl],
                                        et[:, 2 * p : 2 * p + 2, sl],
                                        start=(p == 0),
                                        stop=False,
                                        perf_mode=DR,
                                        skip_group_check=True,
                                    )
                                    nc.tensor.matmul(
                                        ep[:, sl],
                                        w2lo_sb[:, 2 * p : 2 * p + 2, hsl],
                                        et[:, 2 * p : 2 * p + 2, sl],
                                        start=False,
                                        stop=(p == 3),
                                        perf_mode=DR,
                                        skip_group_check=True,
                                    )
                                nc.scalar.activation(
                                    tt_l[:, sl], ep[:, sl], AF.Tanh,
                                    bias=ubias[:, hc, bb : bb + 1],
                                    scale=ACT_SCALE,
                                )
                                nc.vector.scalar_tensor_tensor(
                                    acc[:, sl], tt_l[:, sl],
                                    vcol_sb[:, hc : hc + 1], acc[:, sl],
                                    op0=MUL, op1=ADD,
                                )
                            continue
                        for p in range(4):
                            nc.tensor.matmul(
                                ep[:],
                                w2hi_sb[:, 2 * p : 2 * p + 2, hsl],
                                et[:, 2 * p : 2 * p + 2, :],
                                start=(p == 0),
                                stop=False,
                                perf_mode=DR,
                            )
                            nc.tensor.matmul(
                                ep[:],
                                w2lo_sb[:, 2 * p : 2 * p + 2, hsl],
                                et[:, 2 * p : 2 * p + 2, :],
                                start=False,
                                stop=(p == 3),
                                perf_mode=DR,
                            )
                        if hc == 0 and pending_sum is not None:
                            emit_sum(pending_sum)
                            pending_sum = None
                        postproc(ep, hc)

                pending_sum = (acc, bb, sb)

            emit_sum(pending_sum)

            # ---- softmax over S per batch ---------------------------------
            # no max-subtraction: |scores| <= ||v||_1 (~25), exp() is safely
            # inside fp32 range, and softmax is shift-invariant
            esc = softp.tile([128, S], f32, tag="esc")
            ssum = softp.tile([128, 1], f32, tag="ssum")
            nc.scalar.activation(
                esc[:], scores[:], AF.Exp, bias=0.0, scale=1.0,
                accum_out=ssum[:],
            )
            rsum = softp.tile([128, 1], f32, tag="rsum")
            nc.vector.reciprocal(rsum[:], ssum[:])
            prob = softp.tile([128, S], f32, tag="prob")
            nc.vector.tensor_scalar_mul(prob[:], esc[:], rsum[:])
            for bb in range(B_LOC):
                nc.sync.dma_start(
                    out[bb : bb + 1, :], prob[32 * bb : 32 * bb + 1, :]
                )

    _patch_json(nc)
    return nc


_NC_CACHE = None


def _get_nc():
    global _NC_CACHE
    if _NC_CACHE is None:
        _NC_CACHE = build_kernel()
    return _NC_CACHE


def round_fp32r(x):
    """RNE-round fp32 values to the fp32r grid (drop low 12 mantissa bits)."""
    u = np.ascontiguousarray(x, dtype=np.float32).view(np.uint32)
    r = (u + 0x7FF + ((u >> 12) & 1)) & 0xFFFFF000
    return r.astype(np.uint32).view(np.float32)


def _part_major(x, cols):
    """[KC*128, cols] row-chunked -> [128, KC*cols] partition-major."""
    return np.ascontiguousarray(
        x.reshape(KC, 128, cols).transpose(1, 0, 2).reshape(128, KC * cols)
    )


def shard_inputs(hidden, encoderOutputs, W, b, v):
    """Host-side prep: per-core input dict list."""
    hidden = np.ascontiguousarray(hidden, dtype=np.float32)
    W = np.ascontiguousarray(W, dtype=np.float32)
    b = np.ascontiguousarray(b, dtype=np.float32)
    v = np.ascontiguousarray(v, dtype=np.float32)

    w1t = round_fp32r(np.ascontiguousarray(W[:, :H].T))        # [k, h]
    w1t_pm = _part_major(w1t, H)
    w2s = np.ascontiguousarray(W[:, H:].T) * np.float32(S_W)   # [k, h] x32
    w2hi8 = w2s.astype(F8)
    w2lo8 = (w2s - w2hi8.astype(np.float32)).astype(F8)
    w2hi_pm = _part_major(w2hi8, H)
    w2lo_pm = _part_major(w2lo8, H)
    bcol = np.ascontiguousarray(b.reshape(HC, 128).T)    # [128, hc]
    vcol = np.ascontiguousarray(v.reshape(HC, 128).T)    # [128, hc]
    onesoh = np.zeros((128, B_LOC, 128), np.float32)
    for bb in range(B_LOC):
        onesoh[:, bb, 32 * bb] = 1.0
    onesoh = np.ascontiguousarray(onesoh.reshape(128, B_LOC * 128))

    # [H, B, S] single big transpose, then fp8-quantize once
    encT = np.transpose(np.asarray(encoderOutputs, dtype=np.float32), (2, 1, 0))
    enc8 = (encT * np.float32(S_E)).astype(F8)           # [H, B, S]

    in_maps = []
    for i in range(NCORES):
        b0 = i * B_LOC
        enc_c = np.ascontiguousarray(enc8[:, b0 : b0 + B_LOC, :]).reshape(H, R)
        enc_pm = _part_major(enc_c, R)
        hid_c = hidden[b0 : b0 + B_LOC]                  # [4, H]
        h_t = round_fp32r(
            hid_c.T.reshape(KC, 128, B_LOC).transpose(1, 0, 2).reshape(128, KC * B_LOC)
        )
        in_maps.append(
            {
                "enc_t": enc_pm,
                "h_t": h_t,
                "w1t": w1t_pm,
                "w2hi": w2hi_pm,
                "w2lo": w2lo_pm,
                "bcol": bcol,
                "vcol": vcol,
                "onesoh": onesoh,
            }
        )
    return in_maps


def run(in_maps, trace=False):
    if trace:
        _install_ntff_hook()
    from concourse import bass_utils

    nc = _get_nc()
    res = bass_utils.run_bass_kernel_spmd(
        nc, in_maps, core_ids=list(range(NCORES)), trace=trace
    )
    return res


def kernel(hidden, encoderOutputs, W, b, v):
    in_maps = shard_inputs(hidden, encoderOutputs, W, b, v)
    res = run(in_maps, trace=False)
    outs = [res.results[i]["out"] for i in range(NCORES)]   # each [4, S]
    full = np.concatenate(outs, axis=0)                     # [32, S]
    return full[:, None, :].astype(np.float32)              # [32, 1, S]


# revision 3
# speedup vs baseline: 1.7236x; 1.6826x over previous
"""Bahdanau-attention scoring kernel for Trainium2 (8 NeuronCores).

reference computation:
  enc = transpose(encoderOutputs, (1,0,2))            # [B,S,H]
  energy = tanh(concat([hidden bcast, enc]) @ W^T(2H contraction) + b)
  scores = energy . v ; softmax over S -> [B,1,S]

decomposition used here:
  energy[b,s,h] = tanh( enc[b,s,:] @ W2[h,:] + (hidden[b,:] @ W1[h,:] + b[h]) )
  with W1 = W[:, :H], W2 = W[:, H:].
  The hidden term ("ubias") is per-(b,h), computed once on-device, and folded
  into the tanh as the ScalarE activation's per-partition bias.

mixed-precision main GEMM (per-matmul PE cost is ~constant whether it's a
512-row fp32r or a 1024-row fp8 DoubleRow, so DR doubles throughput):
  - k-chunks 0..1 (256 of 1024): exact fp32r, W2 slice pre-scaled by 512
    (pow2, lossless) so both paths share one PSUM scale
  - k-chunks 2..7 (768 of 1024): fp8 e4m3 (enc x16, W2 x32), DoubleRow
    matmuls over kc-pairs at 0.5 cycles/row
  => 5 matmuls per energy tile instead of 8; quantization noise on 6/8 of K
  gives rel_fro ~1.89e-2 (vs the 2e-2 gate; numpy-predicted and
  HW-reproducible since everything is deterministic).
  The x512 scale is undone by the tanh activation (tanh(psum/512 + ubias)).

sharding: data-parallel over batch B=32 -> 4 batches per core.
Per-core kernel layout:
  - energy tiles [h=128 part, rows=512 free]: 3 DR fp8 + 2 fp32r matmuls
  - tanh fused with per-partition ubias on ScalarE, output bf16
  - v-dot: one fused DVE scalar_tensor_tensor per h-chunk
    (acc = tanh*v_chunk + acc), then one fp32r matmul per row-block with a
    one-hot ones column reduces partitions and lands batch bb's scores on
    psum partition 32*bb
  - softmax over S on a [128, 2048] sbuf tile (4 used partitions), out f32

toolchain notes (this container):
  - walrus here accepts only ONE sync wait per instruction; _split_multiwaits
    rewrites the BIR to single-wait NoOp chains (hooked via nc.to_json_bytes)
  - fp32r matmuls need fp32r-declared producers; inputs are pre-rounded on
    the host (RNE to the fp32r grid) and declared float32r in DRAM
"""

import json
import sys
import types

import ml_dtypes
import numpy as np

H = 1024
S = 2048
B = 32
NCORES = 8
B_LOC = B // NCORES          # 4 batches per core
R = S * B_LOC                # 8192 rows per core (b-major: r = b*S + s)
NBLK = R // 512              # 16 row blocks of 512
KC = H // 128                # 8 contraction chunks
HC = H // 128                # 8 h chunks

KE = 2                       # exact fp32r k-chunks (kc 0..KE-1)
KQ = KC - KE                 # fp8 k-chunks (kc KE..KC-1), must be even
NP8 = KQ // 2                # DoubleRow kc-pairs

S_E = 16.0                   # enc fp8 scale
S_W = 32.0                   # W2 fp8 scale
PSC = S_E * S_W              # common psum scale (fp32r W2 slice pre-scaled)
ACT_SCALE = 1.0 / PSC

F8 = ml_dtypes.float8_e4m3


def _install_ntff_hook():
    """Install antenv.axon_hooks shim so trace=True works under axon."""
    if "antenv.axon_hooks" in sys.modules:
        return
    try:
        from trn_agent_boot.trn_boot import _ntff_profile_via_ctypes

        hook = _ntff_profile_via_ctypes("/opt/axon/libaxon_pjrt.so")
    except Exception:
        hook = None
    mod = types.ModuleType("antenv.axon_hooks")
    mod._hook = hook
    mod.get_axon_ntff_profile_hook = lambda: mod._hook

    def _set(h):
        mod._hook = h

    mod.set_axon_ntff_profile_hook = _set
    sys.modules["antenv.axon_hooks"] = mod


def _split_multiwaits(bir):
    """This walrus build supports one sync wait per instruction: split
    longer on_wait lists into single-wait NoOps on the same engine."""
    for fn in bir["functions"]:
        for blk in fn["blocks"]:
            out = []
            for inst in blk["instructions"]:
                si = inst.get("sync_info")
                ow = (si or {}).get("on_wait") or []
                if len(ow) > 1:
                    for j, w in enumerate(ow[:-1]):
                        out.append(
                            {
                                "debug": inst.get("debug", 0),
                                "engine": inst["engine"],
                                "ins": [],
                                "name": f"{inst['name']}_sw{j}",
                                "opcode": "NoOp",
                                "outs": [],
                                "sync_info": {"on_wait": [w], "on_update": []},
                                "text_hint": "waitsplit",
                            }
                        )
                    si["on_wait"] = [ow[-1]]
                out.append(inst)
            blk["instructions"] = out
    return bir


def _patch_json(nc):
    orig = nc.to_json_bytes

    def patched():
        return json.dumps(_split_multiwaits(json.loads(orig()))).encode()

    nc.to_json_bytes = patched


def build_kernel():
    import concourse.bass as bass
    import concourse.tile as tile
    from concourse import mybir
    from concourse.masks import make_identity

    f32 = mybir.dt.float32
    f32r = mybir.dt.float32r
    bf16 = mybir.dt.bfloat16
    f8 = mybir.dt.float8e4
    AF = mybir.ActivationFunctionType
    DR = mybir.MatmulPerfMode.DoubleRow
    MUL = mybir.AluOpType.mult
    ADD = mybir.AluOpType.add

    nc = bass.Bass("TRN2", target_bir_lowering=False, debug=False, num_devices=1)

    # All big operands are stored partition-major in DRAM ([128, chunks*cols])
    # so each SBUF tile fills with a single (or few) 3D DMA.
    enc8_t = nc.dram_tensor("enc8_t", [128, KQ * R], f8, kind="ExternalInput").ap()
    enc32_t = nc.dram_tensor("enc32_t", [128, KE * R], f32r, kind="ExternalInput").ap()
    h_t = nc.dram_tensor("h_t", [128, KC * B_LOC], f32r, kind="ExternalInput").ap()
    w1t = nc.dram_tensor("w1t", [128, KC * H], f32r, kind="ExternalInput").ap()
    w2r = nc.dram_tensor("w2r", [128, KE * H], f32r, kind="ExternalInput").ap()
    w2hi = nc.dram_tensor("w2hi", [128, KQ * H], f8, kind="ExternalInput").ap()
    bcol = nc.dram_tensor("bcol", [128, HC], f32, kind="ExternalInput").ap()
    vcol = nc.dram_tensor("vcol", [128, HC], f32, kind="ExternalInput").ap()
    onesoh = nc.dram_tensor("onesoh", [128, B_LOC * 128], f32r, kind="ExternalInput").ap()
    out = nc.dram_tensor("out", [B_LOC, S], f32, kind="ExternalOutput").ap()

    enc8_3 = enc8_t.rearrange("p (c r) -> p c r", c=KQ)
    enc32_3 = enc32_t.rearrange("p (c r) -> p c r", c=KE)
    w13 = w1t.rearrange("p (c h) -> p c h", c=KC)
    w2r3 = w2r.rearrange("p (c h) -> p c h", c=KE)
    w2hi3 = w2hi.rearrange("p (c h) -> p c h", c=KQ)

    with tile.TileContext(nc) as tc:
        with (
            tc.tile_pool(name="consts", bufs=1) as consts,
            tc.tile_pool(name="w1p", bufs=1) as w1p,
            tc.tile_pool(name="w2p", bufs=1) as w2p,
            tc.tile_pool(name="encp", bufs=3) as encp,
            tc.tile_pool(name="enc32p", bufs=3) as enc32p,
            tc.tile_pool(name="tanp", bufs=3) as tanp,
            tc.tile_pool(name="accp", bufs=2) as accp,
            tc.tile_pool(name="scorep", bufs=1) as scorep,
            tc.tile_pool(name="softp", bufs=1) as softp,
            tc.tile_pool(name="ep0", bufs=4, space="PSUM") as ep0,      # blk0 pair-outer
            tc.tile_pool(name="epsum", bufs=2, space="PSUM") as epsum,  # blks >= 1
            tc.tile_pool(name="spsum", bufs=2, space="PSUM") as spsum,  # scores + ubias
        ):
            # ---- W2 + enc block 0 on the SP queue ------------------------
            # fp8 pair 0 of everything first so the pair-outer matmuls of
            # block 0 can start while the rest streams in
            w2hi_sb = w2p.tile([128, KQ, H], f8, tag="w2hi_sb")
            w2r_sb = w2p.tile([128, KE, H], f32r, tag="w2r_sb")
            et8_0 = encp.tile([128, KQ, 512], f8, tag="enc8")
            et32_0 = enc32p.tile([128, KE, 512], f32r, tag="enc32")
            nc.sync.dma_start(w2hi_sb[:, 0:2, :], w2hi3[:, 0:2, :])
            nc.sync.dma_start(et8_0[:, 0:2, :], enc8_3[:, 0:2, 0:512])
            nc.sync.dma_start(w2hi_sb[:, 2:KQ, :], w2hi3[:, 2:KQ, :])
            nc.sync.dma_start(et8_0[:, 2:KQ, :], enc8_3[:, 2:KQ, 0:512])
            nc.sync.dma_start(w2r_sb[:], w2r3[:])
            nc.sync.dma_start(et32_0[:], enc32_3[:, :, 0:512])

            # ---- small constants on the gpsimd queue ----------------------
            h_sb = consts.tile([128, KC, B_LOC], f32r, tag="h_sb")
            nc.gpsimd.dma_start(h_sb[:], h_t.rearrange("p (c b) -> p c b", c=KC))
            bcol_sb = consts.tile([128, HC], f32, tag="bcol_sb")
            nc.gpsimd.dma_start(bcol_sb[:], bcol[:])
            vcol_sb = consts.tile([128, HC], f32, tag="vcol_sb")
            nc.gpsimd.dma_start(vcol_sb[:], vcol[:])

            # ones one-hot for the partition-sum matmul: column 32*bb is 1
            ones_oh = consts.tile([128, B_LOC, 128], f32r, tag="ones_oh")
            nc.gpsimd.dma_start(
                ones_oh[:], onesoh.rearrange("p (b m) -> p b m", b=B_LOC)
            )

            # ---- W1T resident, one DMA on the ACT HWDGE queue -------------
            w1sb = w1p.tile([128, KC, H], f32r, tag="w1sb")
            nc.scalar.dma_start(w1sb[:], w13[:])

            # 4x4 identity for the tiny PE transposes of uT
            idt = consts.tile([B_LOC, B_LOC], f32, tag="idt")
            make_identity(nc, idt[:])

            # uT[b, h] = (hidden @ W1^T)[b, h] via wide-N matmuls with the
            # 4-column hidden as the stationary operand (cheap weight loads),
            # then 8 tiny PE transposes to get ubias in [h-part, b] layout
            uts = consts.tile([B_LOC, H], f32, tag="uts")
            ubias = consts.tile([128, HC, B_LOC], f32, tag="ubias")

            def emit_u_half(nh):
                upt = spsum.tile([128, 512], f32, tag="sp")
                for kc in range(KC):
                    nc.tensor.matmul(
                        upt[0:B_LOC, :],
                        h_sb[:, kc, :],
                        w1sb[:, kc, nh * 512 : (nh + 1) * 512],
                        start=(kc == 0),
                        stop=(kc == KC - 1),
                        skip_group_check=True,
                    )
                nc.vector.tensor_copy(
                    uts[0:B_LOC, nh * 512 : (nh + 1) * 512], upt[0:B_LOC, :]
                )
                for hc in range(nh * 4, nh * 4 + 4):
                    trp = spsum.tile([128, 512], f32, tag="sp")
                    nc.tensor.transpose(
                        trp[:, 0:B_LOC],
                        uts[0:B_LOC, hc * 128 : (hc + 1) * 128],
                        idt[:],
                    )
                    nc.vector.tensor_scalar_add(
                        ubias[:, hc, :], trp[:, 0:B_LOC], bcol_sb[:, hc : hc + 1]
                    )

            # ---- main loop over 16 row blocks -----------------------------
            # batch bb's scores live on partition 32*bb
            scores = scorep.tile([128, S], f32, tag="scores")
            nc.vector.memset(scores[:], 0.0)

            pending_sum = None  # (acc tile, bb, sb) awaiting partition-sum MM

            def emit_sum(pending):
                acc, bb, sb = pending
                mw = 32 * bb + 1
                sp = spsum.tile([128, 512], f32, tag="sp")
                nc.tensor.matmul(
                    sp[0:mw, :],
                    ones_oh[:, bb, 0:mw],
                    acc[:],
                    start=True,
                    stop=True,
                    skip_group_check=True,
                )
                nc.vector.tensor_copy(
                    scores[32 * bb : 32 * bb + 1, sb * 512 : (sb + 1) * 512],
                    sp[32 * bb : 32 * bb + 1, :],
                )

            def emit_energy(ep, et8, et32, hsl, sl, skip_check):
                """5 matmuls accumulating one energy psum tile: 3 fp8 DR
                pairs then 2 exact fp32r chunks (x512-scaled weights)."""
                for p in range(NP8):
                    nc.tensor.matmul(
                        ep[:, sl],
                        w2hi_sb[:, 2 * p : 2 * p + 2, hsl],
                        et8[:, 2 * p : 2 * p + 2, sl],
                        start=(p == 0),
                        stop=False,
                        perf_mode=DR,
                        skip_group_check=skip_check,
                    )
                for kc in range(KE):
                    nc.tensor.matmul(
                        ep[:, sl],
                        w2r_sb[:, kc, hsl],
                        et32[:, kc, sl],
                        start=False,
                        stop=(kc == KE - 1),
                        skip_group_check=skip_check,
                    )

            ets = {0: (et8_0, et32_0)}
            for blk in range(NBLK):
                bb = blk // (S // 512)       # batch of this block
                sb = blk % (S // 512)        # block index within the batch
                if blk + 1 < NBLK:
                    et8n = encp.tile([128, KQ, 512], f8, tag="enc8")
                    et32n = enc32p.tile([128, KE, 512], f32r, tag="enc32")
                    csl = slice((blk + 1) * 512, (blk + 2) * 512)
                    nc.sync.dma_start(et8n[:], enc8_3[:, :, csl])
                    nc.sync.dma_start(et32n[:], enc32_3[:, :, csl])
                    ets[blk + 1] = (et8n, et32n)
                et8, et32 = ets.pop(blk)

                acc = accp.tile([128, 512], f32r, tag="acc")

                def postproc(ep, hc):
                    # tanh with fused ubias (undoes the x512 scale), then
                    # one fused DVE op: acc = tanh*v_chunk (+ acc)
                    tt = tanp.tile([128, 512], bf16, tag="tt")
                    nc.scalar.activation(
                        tt[:], ep[:], AF.Tanh,
                        bias=ubias[:, hc, bb : bb + 1], scale=ACT_SCALE,
                    )
                    if hc == 0:
                        nc.vector.tensor_scalar_mul(
                            acc[:], tt[:], vcol_sb[:, hc : hc + 1]
                        )
                    else:
                        nc.vector.scalar_tensor_tensor(
                            acc[:], tt[:], vcol_sb[:, hc : hc + 1], acc[:],
                            op0=MUL, op1=ADD,
                        )

                full = slice(0, 512)
                if blk == 0:
                    # pair-outer halves: PE can start as soon as the first
                    # (w2, enc) kc-pair lands instead of waiting for all 8.
                    # fp8 pairs first (they arrive first), fp32r last.
                    for half in range(2):
                        hcs = range(half * 4, half * 4 + 4)
                        eps = {}
                        for hc in hcs:
                            e0t = ep0.tile([128, 512], f32, tag="e0")
                            eps[hc] = e0t
                        for p in range(NP8):
                            for hc in hcs:
                                hsl = slice(hc * 128, (hc + 1) * 128)
                                nc.tensor.matmul(
                                    eps[hc][:],
                                    w2hi_sb[:, 2 * p : 2 * p + 2, hsl],
                                    et8[:, 2 * p : 2 * p + 2, :],
                                    start=(p == 0),
                                    stop=False,
                                    perf_mode=DR,
                                    skip_group_check=True,
                                )
                        for kc in range(KE):
                            for hc in hcs:
                                hsl = slice(hc * 128, (hc + 1) * 128)
                                nc.tensor.matmul(
                                    eps[hc][:],
                                    w2r_sb[:, kc, hsl],
                                    et32[:, kc, :],
                                    start=False,
                                    stop=(kc == KE - 1),
                                    skip_group_check=True,
                                )
                        emit_u_half(half)
                        for hc in hcs:
                            postproc(eps[hc], hc)
                else:
                    for hc in range(HC):
                        hsl = slice(hc * 128, (hc + 1) * 128)
                        ep = epsum.tile([128, 512], f32, tag="ep")
                        last_chunk = blk == NBLK - 1 and hc == HC - 1
                        if last_chunk:
                            # split the very last energy group into two
                            # 256-column halves so the tanh/v-dot chain (and
                            # with it the final partition-sum) starts half a
                            # group earlier - trims the end-of-stream stall
                            tt_l = tanp.tile([128, 512], bf16, tag="tt")
                            for half in range(2):
                                sl = slice(half * 256, half * 256 + 256)
                                emit_energy(ep, et8, et32, hsl, sl, True)
                                nc.scalar.activation(
                                    tt_l[:, sl], ep[:, sl], AF.Tanh,
                                    bias=ubias[:, hc, bb : bb + 1],
                                    scale=ACT_SCALE,
                                )
                                nc.vector.scalar_tensor_tensor(
                                    acc[:, sl], tt_l[:, sl],
                                    vcol_sb[:, hc : hc + 1], acc[:, sl],
                                    op0=MUL, op1=ADD,
                                )
                            continue
                        emit_energy(ep, et8, et32, hsl, full, False)
                        if hc == 0 and pending_sum is not None:
                            emit_sum(pending_sum)
                            pending_sum = None
                        postproc(ep, hc)

                pending_sum = (acc, bb, sb)

            emit_sum(pending_sum)

            # ---- softmax over S per batch ---------------------------------
            # no max-subtraction: |scores| <= ||v||_1 (~25), exp() is safely
            # inside fp32 range, and softmax is shift-invariant
            esc = softp.tile([128, S], f32, tag="esc")
            ssum = softp.tile([128, 1], f32, tag="ssum")
            nc.scalar.activation(
                esc[:], scores[:], AF.Exp, bias=0.0, scale=1.0,
                accum_out=ssum[:],
            )
            rsum = softp.tile([128, 1], f32, tag="rsum")
            nc.vector.reciprocal(rsum[:], ssum[:])
            prob = softp.tile([128, S], f32, tag="prob")
            nc.vector.tensor_scalar_mul(prob[:], esc[:], rsum[:])
            for bb in range(B_LOC):
                nc.sync.dma_start(
                    out[bb : bb + 1, :], prob[32 * bb : 32 * bb + 1, :]
                )

    _patch_json(nc)
    return nc


_NC_CACHE = None


def _get_nc():
    global _NC_CACHE
    if _NC_CACHE is None:
        _NC_CACHE = build_kernel()
    return _NC_CACHE


def round_fp32r(x):
    """RNE-round fp32 values to the fp32r grid (drop low 12 mantissa bits)."""
    u = np.ascontiguousarray(x, dtype=np.float32).view(np.uint32)
    r = (u + 0x7FF + ((u >> 12) & 1)) & 0xFFFFF000
    return r.astype(np.uint32).view(np.float32)


def _part_major(x, nchunk, cols):
    """[nchunk*128, cols] row-chunked -> [128, nchunk*cols] partition-major."""
    return np.ascontiguousarray(
        x.reshape(nchunk, 128, cols).transpose(1, 0, 2).reshape(128, nchunk * cols)
    )


def shard_inputs(hidden, encoderOutputs, W, b, v):
    """Host-side prep: per-core input dict list."""
    hidden = np.ascontiguousarray(hidden, dtype=np.float32)
    W = np.ascontiguousarray(W, dtype=np.float32)
    b = np.ascontiguousarray(b, dtype=np.float32)
    v = np.ascontiguousarray(v, dtype=np.float32)

    w1t = round_fp32r(np.ascontiguousarray(W[:, :H].T))        # [k, h]
    w1t_pm = _part_major(w1t, KC, H)
    w2t = np.ascontiguousarray(W[:, H:].T)                     # [k, h]
    w2r_pm = _part_major(
        round_fp32r(w2t[: KE * 128] * np.float32(PSC)), KE, H
    )
    w2hi_pm = _part_major(
        (w2t[KE * 128 :] * np.float32(S_W)).astype(F8), KQ, H
    )
    bcol = np.ascontiguousarray(b.reshape(HC, 128).T)    # [128, hc]
    vcol = np.ascontiguousarray(v.reshape(HC, 128).T)    # [128, hc]
    onesoh = np.zeros((128, B_LOC, 128), np.float32)
    for bb in range(B_LOC):
        onesoh[:, bb, 32 * bb] = 1.0
    onesoh = np.ascontiguousarray(onesoh.reshape(128, B_LOC * 128))

    # [H, B, S] single big transpose, then quantize each k-range once
    encT = np.transpose(np.asarray(encoderOutputs, dtype=np.float32), (2, 1, 0))
    enc32 = round_fp32r(encT[: KE * 128])                # [KE*128, B, S] f32r
    enc8 = (encT[KE * 128 :] * np.float32(S_E)).astype(F8)  # [KQ*128, B, S]

    in_maps = []
    for i in range(NCORES):
        b0 = i * B_LOC
        enc8_c = np.ascontiguousarray(enc8[:, b0 : b0 + B_LOC, :]).reshape(
            KQ * 128, R
        )
        enc32_c = np.ascontiguousarray(enc32[:, b0 : b0 + B_LOC, :]).reshape(
            KE * 128, R
        )
        hid_c = hidden[b0 : b0 + B_LOC]                  # [4, H]
        h_t = round_fp32r(
            hid_c.T.reshape(KC, 128, B_LOC).transpose(1, 0, 2).reshape(128, KC * B_LOC)
        )
        in_maps.append(
            {
                "enc8_t": _part_major(enc8_c, KQ, R),
                "enc32_t": _part_major(enc32_c, KE, R),
                "h_t": h_t,
                "w1t": w1t_pm,
                "w2r": w2r_pm,
                "w2hi": w2hi_pm,
                "bcol": bcol,
                "vcol": vcol,
                "onesoh": onesoh,
            }
        )
    return in_maps


def run(in_maps, trace=False):
    if trace:
        _install_ntff_hook()
    from concourse import bass_utils

    nc = _get_nc()
    res = bass_utils.run_bass_kernel_spmd(
        nc, in_maps, core_ids=list(range(NCORES)), trace=trace
    )
    return res


def kernel(hidden, encoderOutputs, W, b, v):
    in_maps = shard_inputs(hidden, encoderOutputs, W, b, v)
    res = run(in_maps, trace=False)
    outs = [res.results[i]["out"] for i in range(NCORES)]   # each [4, S]
    full = np.concatenate(outs, axis=0)                     # [32, S]
    return full[:, None, :].astype(np.float32)              # [32, 1, S]
